# revision 24
# baseline (speedup 1.0000x reference)
"""GCN (3x GCNConv + segment-sum pooling + MLP + log_softmax over graphs)
on 8 Trainium2 NeuronCores.

Strategy: edges sharded by dst range across the 8 cores (graph/data parallel
per the sharding hint). Per conv layer (one SPMD launch): every core
redundantly computes h' = dinv * (h @ W) for all nodes into DRAM (cheap),
then processes its own dst-shard's edges grouped by (dst_tile, src_bucket):
dma_gather (256B rows, 4 SWDGE queues) pulls the messages for one segment
into SBUF; the scatter-add is done ON-CHIP as PE matmuls — for each 128-edge
block a one-hot [128e x 128dst] matrix is built on the Vector engine
(iota == dst_local compare) and matmul-accumulated into a per-dst-tile PSUM
bank. No dma_scatter_add, no DRAM accumulator round trip. Self-loops are
regular edges. The symmetric norm factorizes: msg = h'[src],
out = dinv*(sum msgs) + b.
Pooling = PE matmul with host-built one-hot graph-assignment tiles, emitted
transposed so the final MLP + log_softmax (over the graph axis = free dim)
needs no transposes. Host only does index prep / sharding / reassembly.
"""
import sys
import time
sys.path.insert(0, '/opt/trn_rl_repo')
import numpy as np
from concourse import bass, mybir, bacc, tile
from concourse.bass_utils import run_bass_kernel_spmd

F32 = mybir.dt.float32
I16 = mybir.dt.int16

NC = 8
N = 100000
G = 512
NPER = N // NC              # 12500 dst nodes per core
DT = 98                     # dst tiles of 128 (12544)
NPAD = 100352               # 98 * 1024, = 4 * 25088
BUCKET = 25088              # gather bucket rows (int16-safe)
NB = 4


def _wrap16(ix):
    """[n] -> [128, n//16] int16 (16-partition wrap, replicated to 8 Q7 cores)."""
    w = ix.reshape(-1, 16).T.astype(np.int16)
    return np.tile(w, (8, 1))


def _row_perm(v):
    """Node id -> hp row. The dense phase stores 1024-row batches with
    partition-major layout (partition p holds rows t*1024 + p*8 .. +7), so
    node t*1024 + a*128 + p lands at row t*1024 + p*8 + a. Gather indices
    and dinv_full are permuted to match; dst-side layout is unaffected."""
    t = v >> 10
    r = v & 1023
    return (t << 10) | ((r & 127) << 3) | (r >> 7)


def _prep_edges(src, dst):
    """Group each core's edges (incl. self-loops) by (dst_tile, src_bucket).

    Returns (seg_plan, gflats, dlflats):
      seg_plan: list of (t, b, nblk) — uniform across cores (max count);
        segment length nblk*128, edges beyond a core's count padded with
        gather idx 0 and dst_local -1 (one-hot all-zero column).
      gflats[c]: flat int16 wrapped gather-index stream.
      dlflats[c]: flat f32 dst_local stream (order matches gather output:
        edge k of a segment lands at partition k%128, block k//128).
    """
    per_core = []
    cnt = np.zeros((NC, DT * NB), np.int64)
    for c in range(NC):
        m = (dst >= c * NPER) & (dst < (c + 1) * NPER)
        s = src[m].astype(np.int64)
        d = (dst[m] - c * NPER).astype(np.int64)
        # self-loops: message h'[global own node] -> dst d
        s = np.concatenate([s, c * NPER + np.arange(NPER, dtype=np.int64)])
        d = np.concatenate([d, np.arange(NPER, dtype=np.int64)])
        s = _row_perm(s)  # hp-row space
        key = (d >> 7) * NB + s // BUCKET
        o = np.argsort(key, kind='stable')
        s, d, key = s[o], d[o], key[o]
        cnt[c] = np.bincount(key, minlength=DT * NB)
        per_core.append((s, d, key))
    nmax = cnt.max(axis=0)
    nblk = -(-nmax // 128)
    n16 = -(-nmax // 16) * 16  # gather idx count (16-aligned); rows beyond
    # it in the last 128-block stay garbage but their one-hot cols are 0
    seg_plan = [(k // NB, k % NB, int(nblk[k]), int(n16[k]))
                for k in range(DT * NB) if nblk[k] > 0]
    gflats, dlflats = [], []
    for c in range(NC):
        s, d, key = per_core[c]
        ends = np.cumsum(cnt[c])
        starts = ends - cnt[c]
        gparts, dparts = [], []
        for (t, b, nb, ng) in seg_plan:
            k = t * NB + b
            i0, i1 = starts[k], ends[k]
            n = nb * 128
            gi = np.zeros(ng, np.int64)
            dl = np.full(n, -1.0, np.float32)
            gi[:i1 - i0] = s[i0:i1] - b * BUCKET
            dl[:i1 - i0] = (d[i0:i1] - t * 128).astype(np.float32)
            gparts.append(_wrap16(gi).ravel())
            dparts.append(dl.reshape(nb, 128).T.ravel())  # partition-major
        gflats.append(np.concatenate(gparts))
        dlflats.append(np.concatenate(dparts))
    return seg_plan, gflats, dlflats


def _build_conv(seg_plan, Lg, Ld, relu, pool, repeat=1, skip_edges=False):
    nc = bacc.Bacc(None, target_bir_lowering=False, num_swdge_queues=4)
    hT = nc.declare_dram_parameter("hT", [64, NPAD], F32, isOutput=False)
    Wp = nc.declare_dram_parameter("W", [64, 64], F32, isOutput=False)
    bb = nc.declare_dram_parameter("bb", [128, 64], F32, isOutput=False)
    # dinv pre-rearranged on host: [p, t*8+a] = dinv_hp_row[t*1024+p*8+a]
    dinv_dense = nc.declare_dram_parameter("dinv_dense", [128, NPAD // 128],
                                           F32, isOutput=False)
    # [p, t] = dinv[own_node t*128+p]
    dinv_own = nc.declare_dram_parameter("dinv_own", [128, DT], F32,
                                         isOutput=False)
    gflat = nc.declare_dram_parameter("gflat", [Lg], I16, isOutput=False)
    dlflat = nc.declare_dram_parameter("dlflat", [Ld], F32, isOutput=False)
    iota_p = nc.declare_dram_parameter("iota", [128, 128], F32, isOutput=False)
    if pool:
        Pw = nc.declare_dram_parameter("Pw", [DT, 128, 128], F32, isOutput=False)
        pooledT = nc.declare_dram_parameter("pooledT", [64, 128], F32, isOutput=True)
    out_own = nc.declare_dram_parameter("out_own", [DT * 128, 64], F32, isOutput=True)
    hp = nc.dram_tensor("hp", [NPAD, 64], F32)

    segs_by_tile = [[] for _ in range(DT)]
    for (t, b, nb, ng) in seg_plan:
        segs_by_tile[t].append((b, nb, ng))

    with tile.TileContext(nc) as tc:
        with (
            tc.tile_pool(name="const", bufs=1) as cpool,
            tc.tile_pool(name="lhs", bufs=4) as lpool,
            tc.tile_pool(name="ps", bufs=2, space="PSUM") as pspool,
            tc.tile_pool(name="hv", bufs=4) as hpool,
            tc.tile_pool(name="gi", bufs=12) as gipool,
            tc.tile_pool(name="dl", bufs=12) as dlpool,
            tc.tile_pool(name="ms", bufs=8) as mpool,
            tc.tile_pool(name="oh", bufs=8) as sopool,
            tc.tile_pool(name="pe", bufs=5, space="PSUM") as epool,
            tc.tile_pool(name="fin", bufs=4) as fpool,
            tc.tile_pool(name="pp", bufs=1, space="PSUM") as pppool,
        ):
            W_sb = cpool.tile([64, 64], F32, tag="w")
            nc.sync.dma_start(out=W_sb[:], in_=Wp[:, :])
            bb_sb = cpool.tile([128, 64], F32, tag="bb")
            nc.sync.dma_start(out=bb_sb[:], in_=bb[:, :])
            iota_sb = cpool.tile([128, 128], F32, tag="iota")
            nc.sync.dma_start(out=iota_sb[:], in_=iota_p[:, :])
            dd_sb = cpool.tile([128, NPAD // 128], F32, tag="dd")
            nc.sync.dma_start(out=dd_sb[:], in_=dinv_dense[:, :])
            do_sb = cpool.tile([128, DT], F32, tag="do")
            nc.sync.dma_start(out=do_sb[:], in_=dinv_own[:, :])

            def layer_body():
                # dense phase: h' = dinv * (h @ W) for all nodes, 1024 rows/
                # iter. Stored permuted (partition-major within each batch,
                # see _row_perm); gather indices compensate.
                for t in range(NPAD // 1024):
                    lt = lpool.tile([64, 1024], F32, tag="lt")
                    nc.sync.dma_start(out=lt[:],
                                      in_=hT[:, t * 1024:(t + 1) * 1024])
                    ht8 = hpool.tile([128, 8, 64], F32, tag="ht8")
                    for a in range(8):
                        ps = pspool.tile([128, 64], F32, tag="ps")
                        nc.tensor.matmul(ps[:], lt[:, a * 128:(a + 1) * 128],
                                         W_sb[:], start=True, stop=True)
                        nc.vector.tensor_scalar_mul(
                            ht8[:, a, :], ps[:],
                            dd_sb[:, t * 8 + a:t * 8 + a + 1])
                    nc.sync.dma_start(
                        out=hp[t * 1024:(t + 1) * 1024, :].rearrange(
                            "(p a) f -> p (a f)", p=128),
                        in_=ht8[:])

                # edge phase: per dst tile, gather per-bucket segments and
                # matmul-accumulate one-hot-scattered messages into PSUM
                goff = 0
                doff = 0
                qn = 0
                # prime all ms pool buffers with zeros: 16-aligned gathers
                # leave tail rows of the last block unwritten, and stale
                # SBUF junk there could be NaN (0 * NaN = NaN in the PE).
                # After this, stale content is always finite gathered data.
                maxnb = max(nb for _, _, nb, _ in seg_plan)
                if not skip_edges:
                    for _ in range(8):
                        msz = mpool.tile([128, maxnb, 64], F32, tag="ms")
                        nc.vector.memset(msz[:], 0.0)
                for t in range(DT):
                    segs = segs_by_tile[t]
                    nblk_tot = sum(nb for _, nb, _ in segs)
                    ps_e = epool.tile([128, 64], F32, tag="pse")
                    blk = 0
                    if skip_edges:
                        # timing-only: one dummy matmul so finalize reads a
                        # written PSUM tile
                        nc.tensor.matmul(ps_e[:], iota_sb[:], iota_sb[:, :64],
                                         start=True, stop=True)
                    for (b, nb, ng) in segs:
                        if skip_edges:
                            blk += nb
                            continue
                        n = nb * 128
                        w = ng // 16
                        gi = gipool.tile([128, w], I16, tag="gi")
                        nc.sync.dma_start(
                            out=gi[:],
                            in_=gflat[goff:goff + 128 * w].rearrange(
                                "(p w) -> p w", p=128))
                        goff += 128 * w
                        dl = dlpool.tile([128, nb], F32, tag="dl")
                        nc.sync.dma_start(
                            out=dl[:],
                            in_=dlflat[doff:doff + n].rearrange(
                                "(p a) -> p a", p=128))
                        doff += n
                        ms = mpool.tile([128, nb, 64], F32, tag="ms")
                        nc.gpsimd.dma_gather(
                            out_ap=ms[:],
                            in_ap=hp[b * BUCKET:(b + 1) * BUCKET, :],
                            idxs_ap=gi[:], num_idxs=ng, num_idxs_reg=ng,
                            elem_size=64, single_packet=False, queue_num=qn)
                        qn = (qn + 1) % 4
                        for j in range(nb):
                            S = sopool.tile([128, 128], F32, tag="S")
                            nc.vector.tensor_scalar(S[:], iota_sb[:],
                                                    dl[:, j:j + 1], None,
                                                    mybir.AluOpType.is_equal)
                            nc.tensor.matmul(ps_e[:], S[:], ms[:, j, :],
                                             start=(blk == 0),
                                             stop=(blk == nblk_tot - 1))
                            blk += 1
                    # finalize: out = [relu](dinv * psum + b)
                    ot = fpool.tile([128, 64], F32, tag="ot")
                    nc.vector.tensor_scalar_mul(ot[:], ps_e[:],
                                                do_sb[:, t:t + 1])
                    nc.vector.tensor_add(ot[:], ot[:], bb_sb[:])
                    if relu:
                        nc.scalar.activation(ot[:], ot[:],
                                             mybir.ActivationFunctionType.Relu)
                    nc.sync.dma_start(out=out_own[t * 128:(t + 1) * 128, :],
                                      in_=ot[:])
                    if pool:
                        pt = lpool.tile([128, 128], F32, tag="pt")
                        nc.sync.dma_start(out=pt[:], in_=Pw[t])
                        pp = pppool.tile([64, 128], F32, tag="pp")
                        nc.tensor.matmul(pp[:], ot[:], pt[:],
                                         start=(t == 0), stop=(t == DT - 1))
                if pool:
                    pc = hpool.tile([64, 128], F32, tag="pc")
                    nc.vector.tensor_copy(pc[:], pp[:])
                    nc.sync.dma_start(out=pooledT[:, :], in_=pc[:])

            if repeat == 1:
                layer_body()
            else:
                with tc.For_i(0, repeat):
                    layer_body()
    nc.finalize()
    return nc


def _build_final():
    nc = bacc.Bacc(None, target_bir_lowering=False)
    parts = nc.declare_dram_parameter("parts", [NC, 64, 512], F32, isOutput=False)
    lW1 = nc.declare_dram_parameter("lW1", [64, 32], F32, isOutput=False)
    lb1 = nc.declare_dram_parameter("lb1", [32, 1], F32, isOutput=False)
    lW2 = nc.declare_dram_parameter("lW2", [32, 5], F32, isOutput=False)
    lb2 = nc.declare_dram_parameter("lb2", [5, 1], F32, isOutput=False)
    out = nc.declare_dram_parameter("out", [5, 512], F32, isOutput=True)
    A = mybir.ActivationFunctionType
    with tile.TileContext(nc) as tc:
        with (
            tc.tile_pool(name="sb", bufs=2) as sb,
            tc.tile_pool(name="ps", bufs=2, space="PSUM") as ps,
            tc.tile_pool(name="c1", bufs=1) as c1,
        ):
            pooled = c1.tile([64, 512], F32, tag="pooled")
            nc.sync.dma_start(out=pooled[:], in_=parts[0])
            for i in range(1, NC):
                pt = sb.tile([64, 512], F32, tag="pt")
                nc.sync.dma_start(out=pt[:], in_=parts[i])
                nc.vector.tensor_add(pooled[:], pooled[:], pt[:])
            w1 = c1.tile([64, 32], F32, tag="w1")
            nc.sync.dma_start(out=w1[:], in_=lW1[:, :])
            b1 = c1.tile([32, 1], F32, tag="b1")
            nc.sync.dma_start(out=b1[:], in_=lb1[:, :])
            w2 = c1.tile([32, 5], F32, tag="w2")
            nc.sync.dma_start(out=w2[:], in_=lW2[:, :])
            b2 = c1.tile([5, 1], F32, tag="b2")
            nc.sync.dma_start(out=b2[:], in_=lb2[:, :])

            z1p = ps.tile([32, 512], F32, tag="z1p")
            nc.tensor.matmul(z1p[:], w1[:], pooled[:], start=True, stop=True)
            z1 = sb.tile([32, 512], F32, tag="z1")
            nc.scalar.activation(z1[:], z1p[:], A.Relu, bias=b1[:])
            z2p = ps.tile([5, 512], F32, tag="z2p")
            nc.tensor.matmul(z2p[:], w2[:], z1[:], start=True, stop=True)
            z2 = sb.tile([5, 512], F32, tag="z2")
            nc.vector.tensor_scalar(z2[:], z2p[:], b2[:], None, mybir.AluOpType.add)

            mx = sb.tile([5, 1], F32, tag="mx")
            nc.vector.tensor_reduce(mx[:], z2[:], mybir.AxisListType.X,
                                    mybir.AluOpType.max)
            zc = sb.tile([5, 512], F32, tag="zc")
            nc.vector.tensor_scalar(zc[:], z2[:], mx[:], None,
                                    mybir.AluOpType.subtract)
            ex = sb.tile([5, 512], F32, tag="ex")
            nc.scalar.activation(ex[:], zc[:], A.Exp)
            sm = sb.tile([5, 1], F32, tag="sm")
            nc.vector.tensor_reduce(sm[:], ex[:], mybir.AxisListType.X,
                                    mybir.AluOpType.add)
            ls = sb.tile([5, 1], F32, tag="ls")
            nc.scalar.activation(ls[:], sm[:], A.Ln)
            oo = sb.tile([5, 512], F32, tag="oo")
            nc.vector.tensor_scalar(oo[:], zc[:], ls[:], None,
                                    mybir.AluOpType.subtract)
            nc.sync.dma_start(out=out[:, :], in_=oo[:])
    nc.finalize()
    return nc


def kernel(x, edge_index, batch, W1, b1, W2, b2, W3, b3, lW1, lb1, lW2, lb2,
           _timing_repeat=1):
    kernel.launch_times = []
    x = np.asarray(x, np.float32)
    edge_index = np.asarray(edge_index)
    batch = np.asarray(batch).astype(np.int64)
    src, dst = edge_index[0].astype(np.int64), edge_index[1].astype(np.int64)

    deg = (np.bincount(dst, minlength=N) + 1).astype(np.float64)
    dinv = (1.0 / np.sqrt(deg)).astype(np.float32)
    dinv_pad = np.zeros(NPAD, np.float32)
    dinv_pad[_row_perm(np.arange(N, dtype=np.int64))] = dinv
    dinv_dense = dinv_pad.reshape(NPAD // 1024, 128, 8).transpose(
        1, 0, 2).reshape(128, NPAD // 128).copy()

    seg_plan, gflats, dlflats = _prep_edges(src, dst)
    Lg = len(gflats[0])
    Ld = len(dlflats[0])
    iota_np = np.tile(np.arange(128, dtype=np.float32), (128, 1)).copy()

    def pad_w(w, fin):
        w = np.asarray(w, np.float32)
        wp = np.zeros((64, 64), np.float32)
        wp[:fin, :w.shape[1]] = w
        return wp

    def pad_b(b):
        bp = np.zeros(64, np.float32)
        b = np.asarray(b, np.float32)
        bp[:len(b)] = b
        return np.tile(bp, (128, 1))

    conv_r = _build_conv(seg_plan, Lg, Ld, relu=True, pool=False,
                         repeat=_timing_repeat)
    conv_p = _build_conv(seg_plan, Lg, Ld, relu=False, pool=True,
                         repeat=_timing_repeat)

    # per-core pooling one-hots + graph window bases
    g0s, Pws = [], []
    for c in range(NC):
        gb = batch[c * NPER:(c + 1) * NPER]
        g0 = int(gb[0])
        g0s.append(g0)
        P = np.zeros((DT, 128, 128), np.float32)
        for t in range(DT):
            for j in range(128):
                node = t * 128 + j
                if node < NPER:
                    col = int(gb[node]) - g0
                    if 0 <= col < 128:
                        P[t, j, col] = 1.0
        Pws.append(P)

    def run_conv(ncc, hT, Wp, bp, pool):
        ins = []
        for c in range(NC):
            m = {
                "hT": hT, "W": Wp, "bb": bp,
                "dinv_dense": dinv_dense,
                "dinv_own": np.pad(dinv[c * NPER:(c + 1) * NPER],
                                   (0, DT * 128 - NPER)).reshape(
                                       DT, 128).T.copy(),
                "gflat": gflats[c], "dlflat": dlflats[c],
                "iota": iota_np,
            }
            if pool:
                m["Pw"] = Pws[c]
            ins.append(m)
        t0 = time.perf_counter()
        res = run_bass_kernel_spmd(ncc, ins, core_ids=list(range(NC)))
        kernel.launch_times.append(time.perf_counter() - t0)
        h = np.zeros((NPAD, 64), np.float32)
        for c in range(NC):
            h[c * NPER:(c + 1) * NPER] = res.results[c]["out_own"][:NPER]
        pooledT = None
        if pool:
            pooledT = np.zeros((NC, 64, 512), np.float32)
            for c in range(NC):
                w = min(128, 512 - g0s[c])
                pooledT[c][:, g0s[c]:g0s[c] + w] = res.results[c]["pooledT"][:, :w]
        return h, pooledT

    hT0 = np.zeros((64, NPAD), np.float32)
    hT0[:3, :N] = x.T
    h1, _ = run_conv(conv_r, hT0, pad_w(W1, 3), pad_b(b1), False)
    h2, _ = run_conv(conv_r, h1.T.copy(), pad_w(W2, 32), pad_b(b2), False)
    _, pooledT = run_conv(conv_p, h2.T.copy(), pad_w(W3, 64), pad_b(b3), True)

    final = _build_final()
    fin = {
        "parts": pooledT,
        "lW1": np.asarray(lW1, np.float32),
        "lb1": np.asarray(lb1, np.float32).reshape(32, 1),
        "lW2": np.asarray(lW2, np.float32),
        "lb2": np.asarray(lb2, np.float32).reshape(5, 1),
    }
    t0 = time.perf_counter()
    res = run_bass_kernel_spmd(final, [fin] * NC, core_ids=list(range(NC)))
    kernel.launch_times.append(time.perf_counter() - t0)
    return np.ascontiguousarray(res.results[0]["out"].T).astype(np.float32)


# revision 26
# speedup vs baseline: 26.9289x; 26.9289x over previous
"""GCN (3x GCNConv + segment-sum pooling + MLP + log_softmax over graphs)
on 8 Trainium2 NeuronCores.

Strategy: edges sharded by dst range across the 8 cores (graph/data parallel
per the sharding hint). Per conv layer (one SPMD launch): every core
redundantly computes h' = dinv * (h @ W) for all nodes into DRAM (cheap),
then processes its own dst-shard's edges grouped by (dst_tile, src_bucket):
dma_gather (256B rows, 4 SWDGE queues) pulls the messages for one segment
into SBUF; the scatter-add is done ON-CHIP as PE matmuls — for each 128-edge
block a one-hot [128e x 128dst] matrix is built on the Vector engine
(iota == dst_local compare) and matmul-accumulated into a per-dst-tile PSUM
bank. No dma_scatter_add, no DRAM accumulator round trip. Self-loops are
regular edges. The symmetric norm factorizes: msg = h'[src],
out = dinv*(sum msgs) + b.
Pooling = PE matmul with host-built one-hot graph-assignment tiles, emitted
transposed so the final MLP + log_softmax (over the graph axis = free dim)
needs no transposes. Host only does index prep / sharding / reassembly.
"""
import sys
import time
sys.path.insert(0, '/opt/trn_rl_repo')
import numpy as np
from concourse import bass, mybir, bacc, tile
from concourse.bass_utils import run_bass_kernel_spmd

F32 = mybir.dt.float32
I16 = mybir.dt.int16

NC = 8
N = 100000
G = 512
NPER = N // NC              # 12500 dst nodes per core
DT = 98                     # dst tiles of 128 (12544)
NPAD = 100352               # 98 * 1024, = 4 * 25088
BUCKET = 25088              # gather bucket rows (int16-safe)
NB = 4


def _wrap16(ix):
    """[n] -> [128, n//16] int16 (16-partition wrap, replicated to 8 Q7 cores)."""
    w = ix.reshape(-1, 16).T.astype(np.int16)
    return np.tile(w, (8, 1))


def _row_perm(v):
    """Node id -> hp row. The dense phase stores 1024-row batches with
    partition-major layout (partition p holds rows t*1024 + p*8 .. +7), so
    node t*1024 + a*128 + p lands at row t*1024 + p*8 + a. Gather indices
    and dinv_full are permuted to match; dst-side layout is unaffected."""
    t = v >> 10
    r = v & 1023
    return (t << 10) | ((r & 127) << 3) | (r >> 7)


def _prep_edges(src, dst, pad128=False):
    """Group each core's edges (incl. self-loops) by (dst_tile, src_bucket).

    Returns (seg_plan, gflats, dlflats):
      seg_plan: list of (t, b, nblk) — uniform across cores (max count);
        segment length nblk*128, edges beyond a core's count padded with
        gather idx 0 and dst_local -1 (one-hot all-zero column).
      gflats[c]: flat int16 wrapped gather-index stream.
      dlflats[c]: flat f32 dst_local stream (order matches gather output:
        edge k of a segment lands at partition k%128, block k//128).
    """
    per_core = []
    cnt = np.zeros((NC, DT * NB), np.int64)
    for c in range(NC):
        m = (dst >= c * NPER) & (dst < (c + 1) * NPER)
        s = src[m].astype(np.int64)
        d = (dst[m] - c * NPER).astype(np.int64)
        # self-loops: message h'[global own node] -> dst d
        s = np.concatenate([s, c * NPER + np.arange(NPER, dtype=np.int64)])
        d = np.concatenate([d, np.arange(NPER, dtype=np.int64)])
        s = _row_perm(s)  # hp-row space
        key = (d >> 7) * NB + s // BUCKET
        o = np.argsort(key, kind='stable')
        s, d, key = s[o], d[o], key[o]
        cnt[c] = np.bincount(key, minlength=DT * NB)
        per_core.append((s, d, key))
    nmax = cnt.max(axis=0)
    nblk = -(-nmax // 128)
    if pad128:
        nmax = nblk * 128
    n16 = -(-nmax // 16) * 16  # gather idx count (16-aligned); rows beyond
    # it in the last 128-block stay garbage but their one-hot cols are 0
    seg_plan = [(k // NB, k % NB, int(nblk[k]), int(n16[k]))
                for k in range(DT * NB) if nblk[k] > 0]
    gflats, dlflats = [], []
    for c in range(NC):
        s, d, key = per_core[c]
        ends = np.cumsum(cnt[c])
        starts = ends - cnt[c]
        gparts, dparts = [], []
        for (t, b, nb, ng) in seg_plan:
            k = t * NB + b
            i0, i1 = starts[k], ends[k]
            n = nb * 128
            gi = np.zeros(ng, np.int64)
            dl = np.full(n, -1.0, np.float32)
            gi[:i1 - i0] = s[i0:i1] - b * BUCKET
            dl[:i1 - i0] = (d[i0:i1] - t * 128).astype(np.float32)
            gparts.append(_wrap16(gi).ravel())
            dparts.append(dl.reshape(nb, 128).T.ravel())  # partition-major
        gflats.append(np.concatenate(gparts))
        dlflats.append(np.concatenate(dparts))
    return seg_plan, gflats, dlflats


def _build_conv(seg_plan, Lg, Ld, relu, pool, repeat=1, skip_edges=False):
    nc = bacc.Bacc(None, target_bir_lowering=False, num_swdge_queues=4)
    hT = nc.declare_dram_parameter("hT", [64, NPAD], F32, isOutput=False)
    Wp = nc.declare_dram_parameter("W", [64, 64], F32, isOutput=False)
    bb = nc.declare_dram_parameter("bb", [128, 64], F32, isOutput=False)
    # dinv pre-rearranged on host: [p, t*8+a] = dinv_hp_row[t*1024+p*8+a]
    dinv_dense = nc.declare_dram_parameter("dinv_dense", [128, NPAD // 128],
                                           F32, isOutput=False)
    # [p, t] = dinv[own_node t*128+p]
    dinv_own = nc.declare_dram_parameter("dinv_own", [128, DT], F32,
                                         isOutput=False)
    gflat = nc.declare_dram_parameter("gflat", [Lg], I16, isOutput=False)
    dlflat = nc.declare_dram_parameter("dlflat", [Ld], F32, isOutput=False)
    iota_p = nc.declare_dram_parameter("iota", [128, 128], F32, isOutput=False)
    if pool:
        Pw = nc.declare_dram_parameter("Pw", [DT, 128, 128], F32, isOutput=False)
        pooledT = nc.declare_dram_parameter("pooledT", [64, 128], F32, isOutput=True)
    out_own = nc.declare_dram_parameter("out_own", [DT * 128, 64], F32, isOutput=True)
    hp = nc.dram_tensor("hp", [NPAD, 64], F32)

    segs_by_tile = [[] for _ in range(DT)]
    for (t, b, nb, ng) in seg_plan:
        segs_by_tile[t].append((b, nb, ng))

    with tile.TileContext(nc) as tc:
        with (
            tc.tile_pool(name="const", bufs=1) as cpool,
            tc.tile_pool(name="lhs", bufs=4) as lpool,
            tc.tile_pool(name="ps", bufs=2, space="PSUM") as pspool,
            tc.tile_pool(name="hv", bufs=4) as hpool,
            tc.tile_pool(name="gi", bufs=12) as gipool,
            tc.tile_pool(name="dl", bufs=12) as dlpool,
            tc.tile_pool(name="ms", bufs=8) as mpool,
            tc.tile_pool(name="oh", bufs=8) as sopool,
            tc.tile_pool(name="pe", bufs=5, space="PSUM") as epool,
            tc.tile_pool(name="fin", bufs=4) as fpool,
            tc.tile_pool(name="pp", bufs=1, space="PSUM") as pppool,
        ):
            W_sb = cpool.tile([64, 64], F32, tag="w")
            nc.sync.dma_start(out=W_sb[:], in_=Wp[:, :])
            bb_sb = cpool.tile([128, 64], F32, tag="bb")
            nc.sync.dma_start(out=bb_sb[:], in_=bb[:, :])
            iota_sb = cpool.tile([128, 128], F32, tag="iota")
            nc.sync.dma_start(out=iota_sb[:], in_=iota_p[:, :])
            dd_sb = cpool.tile([128, NPAD // 128], F32, tag="dd")
            nc.sync.dma_start(out=dd_sb[:], in_=dinv_dense[:, :])
            do_sb = cpool.tile([128, DT], F32, tag="do")
            nc.sync.dma_start(out=do_sb[:], in_=dinv_own[:, :])

            def layer_body():
                # dense phase: h' = dinv * (h @ W) for all nodes, 1024 rows/
                # iter. Stored permuted (partition-major within each batch,
                # see _row_perm); gather indices compensate.
                for t in range(NPAD // 1024):
                    lt = lpool.tile([64, 1024], F32, tag="lt")
                    nc.sync.dma_start(out=lt[:],
                                      in_=hT[:, t * 1024:(t + 1) * 1024])
                    ht8 = hpool.tile([128, 8, 64], F32, tag="ht8")
                    for a in range(8):
                        ps = pspool.tile([128, 64], F32, tag="ps")
                        nc.tensor.matmul(ps[:], lt[:, a * 128:(a + 1) * 128],
                                         W_sb[:], start=True, stop=True)
                        nc.vector.tensor_scalar_mul(
                            ht8[:, a, :], ps[:],
                            dd_sb[:, t * 8 + a:t * 8 + a + 1])
                    nc.sync.dma_start(
                        out=hp[t * 1024:(t + 1) * 1024, :].rearrange(
                            "(p a) f -> p (a f)", p=128),
                        in_=ht8[:])

                # edge phase: per dst tile, gather per-bucket segments and
                # matmul-accumulate one-hot-scattered messages into PSUM
                goff = 0
                doff = 0
                qn = 0
                # prime all ms pool buffers with zeros: 16-aligned gathers
                # leave tail rows of the last block unwritten, and stale
                # SBUF junk there could be NaN (0 * NaN = NaN in the PE).
                # After this, stale content is always finite gathered data.
                maxnb = max(nb for _, _, nb, _ in seg_plan)
                if not skip_edges:
                    for _ in range(8):
                        msz = mpool.tile([128, maxnb, 64], F32, tag="ms")
                        nc.vector.memset(msz[:], 0.0)
                for t in range(DT):
                    segs = segs_by_tile[t]
                    nblk_tot = sum(nb for _, nb, _ in segs)
                    ps_e = epool.tile([128, 64], F32, tag="pse")
                    blk = 0
                    if skip_edges:
                        # timing-only: one dummy matmul so finalize reads a
                        # written PSUM tile
                        nc.tensor.matmul(ps_e[:], iota_sb[:], iota_sb[:, :64],
                                         start=True, stop=True)
                    for (b, nb, ng) in segs:
                        if skip_edges:
                            blk += nb
                            continue
                        n = nb * 128
                        w = ng // 16
                        gi = gipool.tile([128, w], I16, tag="gi")
                        nc.sync.dma_start(
                            out=gi[:],
                            in_=gflat[goff:goff + 128 * w].rearrange(
                                "(p w) -> p w", p=128))
                        goff += 128 * w
                        dl = dlpool.tile([128, nb], F32, tag="dl")
                        nc.sync.dma_start(
                            out=dl[:],
                            in_=dlflat[doff:doff + n].rearrange(
                                "(p a) -> p a", p=128))
                        doff += n
                        ms = mpool.tile([128, nb, 64], F32, tag="ms")
                        nc.gpsimd.dma_gather(
                            out_ap=ms[:],
                            in_ap=hp[b * BUCKET:(b + 1) * BUCKET, :],
                            idxs_ap=gi[:], num_idxs=ng, num_idxs_reg=ng,
                            elem_size=64, single_packet=False, queue_num=qn)
                        qn = (qn + 1) % 4
                        for j in range(nb):
                            S = sopool.tile([128, 128], F32, tag="S")
                            nc.vector.tensor_scalar(S[:], iota_sb[:],
                                                    dl[:, j:j + 1], None,
                                                    mybir.AluOpType.is_equal)
                            nc.tensor.matmul(ps_e[:], S[:], ms[:, j, :],
                                             start=(blk == 0),
                                             stop=(blk == nblk_tot - 1))
                            blk += 1
                    # finalize: out = [relu](dinv * psum + b)
                    ot = fpool.tile([128, 64], F32, tag="ot")
                    nc.vector.tensor_scalar_mul(ot[:], ps_e[:],
                                                do_sb[:, t:t + 1])
                    nc.vector.tensor_add(ot[:], ot[:], bb_sb[:])
                    if relu:
                        nc.scalar.activation(ot[:], ot[:],
                                             mybir.ActivationFunctionType.Relu)
                    nc.sync.dma_start(out=out_own[t * 128:(t + 1) * 128, :],
                                      in_=ot[:])
                    if pool:
                        pt = lpool.tile([128, 128], F32, tag="pt")
                        nc.sync.dma_start(out=pt[:], in_=Pw[t])
                        pp = pppool.tile([64, 128], F32, tag="pp")
                        nc.tensor.matmul(pp[:], ot[:], pt[:],
                                         start=(t == 0), stop=(t == DT - 1))
                if pool:
                    pc = hpool.tile([64, 128], F32, tag="pc")
                    nc.vector.tensor_copy(pc[:], pp[:])
                    nc.sync.dma_start(out=pooledT[:, :], in_=pc[:])

            if repeat == 1:
                layer_body()
            else:
                with tc.For_i(0, repeat):
                    layer_body()
    nc.finalize()
    return nc


def _build_final():
    nc = bacc.Bacc(None, target_bir_lowering=False)
    parts = nc.declare_dram_parameter("parts", [NC, 64, 512], F32, isOutput=False)
    lW1 = nc.declare_dram_parameter("lW1", [64, 32], F32, isOutput=False)
    lb1 = nc.declare_dram_parameter("lb1", [32, 1], F32, isOutput=False)
    lW2 = nc.declare_dram_parameter("lW2", [32, 5], F32, isOutput=False)
    lb2 = nc.declare_dram_parameter("lb2", [5, 1], F32, isOutput=False)
    out = nc.declare_dram_parameter("out", [5, 512], F32, isOutput=True)
    A = mybir.ActivationFunctionType
    with tile.TileContext(nc) as tc:
        with (
            tc.tile_pool(name="sb", bufs=2) as sb,
            tc.tile_pool(name="ps", bufs=2, space="PSUM") as ps,
            tc.tile_pool(name="c1", bufs=1) as c1,
        ):
            pooled = c1.tile([64, 512], F32, tag="pooled")
            nc.sync.dma_start(out=pooled[:], in_=parts[0])
            for i in range(1, NC):
                pt = sb.tile([64, 512], F32, tag="pt")
                nc.sync.dma_start(out=pt[:], in_=parts[i])
                nc.vector.tensor_add(pooled[:], pooled[:], pt[:])
            w1 = c1.tile([64, 32], F32, tag="w1")
            nc.sync.dma_start(out=w1[:], in_=lW1[:, :])
            b1 = c1.tile([32, 1], F32, tag="b1")
            nc.sync.dma_start(out=b1[:], in_=lb1[:, :])
            w2 = c1.tile([32, 5], F32, tag="w2")
            nc.sync.dma_start(out=w2[:], in_=lW2[:, :])
            b2 = c1.tile([5, 1], F32, tag="b2")
            nc.sync.dma_start(out=b2[:], in_=lb2[:, :])

            z1p = ps.tile([32, 512], F32, tag="z1p")
            nc.tensor.matmul(z1p[:], w1[:], pooled[:], start=True, stop=True)
            z1 = sb.tile([32, 512], F32, tag="z1")
            nc.scalar.activation(z1[:], z1p[:], A.Relu, bias=b1[:])
            z2p = ps.tile([5, 512], F32, tag="z2p")
            nc.tensor.matmul(z2p[:], w2[:], z1[:], start=True, stop=True)
            z2 = sb.tile([5, 512], F32, tag="z2")
            nc.vector.tensor_scalar(z2[:], z2p[:], b2[:], None, mybir.AluOpType.add)

            mx = sb.tile([5, 1], F32, tag="mx")
            nc.vector.tensor_reduce(mx[:], z2[:], mybir.AxisListType.X,
                                    mybir.AluOpType.max)
            zc = sb.tile([5, 512], F32, tag="zc")
            nc.vector.tensor_scalar(zc[:], z2[:], mx[:], None,
                                    mybir.AluOpType.subtract)
            ex = sb.tile([5, 512], F32, tag="ex")
            nc.scalar.activation(ex[:], zc[:], A.Exp)
            sm = sb.tile([5, 1], F32, tag="sm")
            nc.vector.tensor_reduce(sm[:], ex[:], mybir.AxisListType.X,
                                    mybir.AluOpType.add)
            ls = sb.tile([5, 1], F32, tag="ls")
            nc.scalar.activation(ls[:], sm[:], A.Ln)
            oo = sb.tile([5, 512], F32, tag="oo")
            nc.vector.tensor_scalar(oo[:], zc[:], ls[:], None,
                                    mybir.AluOpType.subtract)
            nc.sync.dma_start(out=out[:, :], in_=oo[:])
    nc.finalize()
    return nc


def kernel(x, edge_index, batch, W1, b1, W2, b2, W3, b3, lW1, lb1, lW2, lb2,
           _timing_repeat=1):
    kernel.launch_times = []
    x = np.asarray(x, np.float32)
    edge_index = np.asarray(edge_index)
    batch = np.asarray(batch).astype(np.int64)
    src, dst = edge_index[0].astype(np.int64), edge_index[1].astype(np.int64)

    deg = (np.bincount(dst, minlength=N) + 1).astype(np.float64)
    dinv = (1.0 / np.sqrt(deg)).astype(np.float32)
    dinv_pad = np.zeros(NPAD, np.float32)
    dinv_pad[_row_perm(np.arange(N, dtype=np.int64))] = dinv
    dinv_dense = dinv_pad.reshape(NPAD // 1024, 128, 8).transpose(
        1, 0, 2).reshape(128, NPAD // 128).copy()

    seg_plan, gflats, dlflats = _prep_edges(src, dst)
    Lg = len(gflats[0])
    Ld = len(dlflats[0])
    iota_np = np.tile(np.arange(128, dtype=np.float32), (128, 1)).copy()

    def pad_w(w, fin):
        w = np.asarray(w, np.float32)
        wp = np.zeros((64, 64), np.float32)
        wp[:fin, :w.shape[1]] = w
        return wp

    def pad_b(b):
        bp = np.zeros(64, np.float32)
        b = np.asarray(b, np.float32)
        bp[:len(b)] = b
        return np.tile(bp, (128, 1))

    conv_r = _build_conv(seg_plan, Lg, Ld, relu=True, pool=False,
                         repeat=_timing_repeat)
    conv_p = _build_conv(seg_plan, Lg, Ld, relu=False, pool=True,
                         repeat=_timing_repeat)

    # per-core pooling one-hots + graph window bases
    g0s, Pws = [], []
    for c in range(NC):
        gb = batch[c * NPER:(c + 1) * NPER]
        g0 = int(gb[0])
        g0s.append(g0)
        P = np.zeros((DT, 128, 128), np.float32)
        for t in range(DT):
            for j in range(128):
                node = t * 128 + j
                if node < NPER:
                    col = int(gb[node]) - g0
                    if 0 <= col < 128:
                        P[t, j, col] = 1.0
        Pws.append(P)

    def run_conv(ncc, hT, Wp, bp, pool):
        ins = []
        for c in range(NC):
            m = {
                "hT": hT, "W": Wp, "bb": bp,
                "dinv_dense": dinv_dense,
                "dinv_own": np.pad(dinv[c * NPER:(c + 1) * NPER],
                                   (0, DT * 128 - NPER)).reshape(
                                       DT, 128).T.copy(),
                "gflat": gflats[c], "dlflat": dlflats[c],
                "iota": iota_np,
            }
            if pool:
                m["Pw"] = Pws[c]
            ins.append(m)
        t0 = time.perf_counter()
        res = run_bass_kernel_spmd(ncc, ins, core_ids=list(range(NC)))
        kernel.launch_times.append(time.perf_counter() - t0)
        h = np.zeros((NPAD, 64), np.float32)
        for c in range(NC):
            h[c * NPER:(c + 1) * NPER] = res.results[c]["out_own"][:NPER]
        pooledT = None
        if pool:
            pooledT = np.zeros((NC, 64, 512), np.float32)
            for c in range(NC):
                w = min(128, 512 - g0s[c])
                pooledT[c][:, g0s[c]:g0s[c] + w] = res.results[c]["pooledT"][:, :w]
        return h, pooledT

    hT0 = np.zeros((64, NPAD), np.float32)
    hT0[:3, :N] = x.T
    h1, _ = run_conv(conv_r, hT0, pad_w(W1, 3), pad_b(b1), False)
    h2, _ = run_conv(conv_r, h1.T.copy(), pad_w(W2, 32), pad_b(b2), False)
    _, pooledT = run_conv(conv_p, h2.T.copy(), pad_w(W3, 64), pad_b(b3), True)

    final = _build_final()
    fin = {
        "parts": pooledT,
        "lW1": np.asarray(lW1, np.float32),
        "lb1": np.asarray(lb1, np.float32).reshape(32, 1),
        "lW2": np.asarray(lW2, np.float32),
        "lb2": np.asarray(lb2, np.float32).reshape(5, 1),
    }
    t0 = time.perf_counter()
    res = run_bass_kernel_spmd(final, [fin] * NC, core_ids=list(range(NC)))
    kernel.launch_times.append(time.perf_counter() - t0)
    return np.ascontiguousarray(res.results[0]["out"].T).astype(np.float32)
